# revision 7
# baseline (speedup 1.0000x reference)
"""PointHop octant-binning kernel for TRN2 (8 NeuronCores, B-sharded).

v4 design (all rates HW-measured):
- masks via tensor_scalar is_gt (bf16 4x DVE mode)
- masked products via tensor_tensor mult (bf16 2x), broadcast-merged
- half the lattice via relu: relu(m*x) = m*relu(x) (tensor_scalar_max, 4x)
- k-reduction via binary halving tree: fp16 TT adds (2x) for the first 3
  levels, f32 for the last 3 (precision), one instruction per level for
  all arrays; counts subtree runs on GPSIMD in parallel
- epilogue (Moebius butterfly, counts, means, std) batched over 4 slabs

MEGA m-layout: m0-23 value lattice (q-major, u=4bx+2by+bz), m24-26
squares, m27-33 count masks u1..u7.
"""

import os
from contextlib import ExitStack

import numpy as np

if "axon" not in os.environ.get("JAX_PLATFORMS", "axon"):
    os.environ.pop("JAX_PLATFORMS", None)

import concourse.bass as bass
import concourse.bacc as bacc
import concourse.tile as tile
from concourse import mybir
from concourse.bass_utils import run_bass_kernel_spmd

B, C, N, K = 32, 3, 8192, 64
NCORES = 8
BL = B // NCORES          # 4 batches per core
PART = 128
TG = 8                    # groups per partition per slab
SLAB = PART * TG          # 1024 groups per slab
NSLAB = BL * N // SLAB    # 32 slabs per core
SB = 8                    # slabs per epilogue batch
NBATCH = NSLAB // SB
FOUT = 30
SEC = TG * K              # 512 elems per section
V32 = SB * TG             # batched (s, t) dim
NMV = 27                  # value+square arrays (DVE tree)
NMC = 7                   # count arrays (GPSIMD tree)
NM = NMV + NMC

AL = mybir.AluOpType
AF = mybir.ActivationFunctionType
F32 = mybir.dt.float32
FP16 = mybir.dt.float16


def _build_kernel(nc: bass.Bass):
    gx = nc.dram_tensor("gx", [BL, C, N, K], F32, kind="ExternalInput")
    nx = nc.dram_tensor("nx", [BL, N, C], F32, kind="ExternalInput")
    out = nc.dram_tensor("out", [BL, N, FOUT], F32, kind="ExternalOutput")

    with tile.TileContext(nc) as tc, ExitStack() as ctx:
        vpool = ctx.enter_context(tc.tile_pool(name="v", bufs=3))
        mpool = ctx.enter_context(tc.tile_pool(name="m", bufs=2))
        epool = ctx.enter_context(tc.tile_pool(name="e", bufs=2))

        ts = nc.vector.tensor_scalar
        tt = nc.vector.tensor_tensor
        act = nc.scalar.activation

        for batch in range(NBATCH):
            EP = epool.tile([PART, NM * V32], F32, name="EP")   # (m, s, t)
            CT = epool.tile([PART, 8 * V32], F32, name="CT")    # (u, s, t)
            CIN = epool.tile([PART, SB * TG * C], F32, name="CIN")
            O = epool.tile([PART, SB * TG * FOUT], F32, name="O")
            nc.gpsimd.memset(CT[:, 0:V32], float(K))  # count u0 = K

            ep4 = EP[:].rearrange("p (m s t) -> p m s t", m=NM, s=SB)
            ct4 = CT[:].rearrange("p (u s t) -> p u s t", u=8, s=SB)

            for sl in range(SB):
                slab = batch * SB + sl
                b, s = divmod(slab, N // SLAB)
                n0 = s * SLAB

                V = vpool.tile([PART, C * SEC], F32, name="V")
                nc.sync.dma_start(
                    out=V[:].rearrange("p (c t k) -> p c t k", c=C, t=TG),
                    in_=gx[b, :, n0:n0 + SLAB, :].rearrange(
                        "c (p t) k -> p c t k", p=PART, t=TG))
                nc.sync.dma_start(
                    out=CIN[:, sl * TG * C:(sl + 1) * TG * C].rearrange(
                        "p (t c) -> p t c", t=TG),
                    in_=nx[b, n0:n0 + SLAB, :].rearrange(
                        "(p t) c -> p t c", p=PART, t=TG))

                MEGA = mpool.tile([PART, NM * SEC], FP16, name="MEGA")

                def M(m, nsec=1):
                    return MEGA[:, m * SEC:(m + nsec) * SEC]

                vq = V[:].rearrange("p (c s) -> p c s", c=C)
                mq = MEGA[:, 0:24 * SEC].rearrange(
                    "p (q u s) -> p q u s", q=3, u=8)
                act(mq[:, :, 0, :], vq, AF.Copy)      # cast x,y,z -> u0
                act(M(24, 3), V[:], AF.Square)        # squares m24-26

                # count masks (4x): [m27,m28]=[mz,my] via one
                # negative-stride op; m30=mx
                ts(MEGA[:, 27 * SEC:29 * SEC].rearrange(
                       "p (a s) -> p a s", a=2),
                   MEGA[:, 0:24 * SEC].rearrange(
                       "p (a b s) -> p a b s", a=3, b=8)[:, 2:0:-1, 0, :],
                   0.0, None, AL.is_gt)
                ts(M(30), M(0), 0.0, None, AL.is_gt)
                def bcast(ap, n):
                    return ap[:, None, :].broadcast_to([PART, n, SEC])

                # composites: m29=my*mz, then [m31,m32,m33] =
                # [mz,my,myz] * mx (one broadcast TT)
                tt(M(29), M(27), M(28), AL.mult)
                tt(M(31, 3).rearrange("p (a s) -> p a s", a=3),
                   MEGA[:, 27 * SEC:30 * SEC].rearrange(
                       "p (a s) -> p a s", a=3),
                   bcast(M(30), 3), AL.mult)

                # value products via TT (2x), u = 4bx+2by+bz
                # s1: mz * [x, y] -> q0u1(m1), q1u1(m9)
                d1 = MEGA[:, SEC:17 * SEC].rearrange(
                    "p (a b s) -> p a b s", a=2, b=8)[:, :, 0, :]
                s1 = MEGA[:, 0:16 * SEC].rearrange(
                    "p (a b s) -> p a b s", a=2, b=8)[:, :, 0, :]
                tt(d1, s1, bcast(M(27), 2), AL.mult)
                # s2: my * [x, mz*x] -> m2, m3
                tt(M(2, 2).rearrange("p (a s) -> p a s", a=2),
                   M(0, 2).rearrange("p (a s) -> p a s", a=2),
                   bcast(M(28), 2), AL.mult)
                # s3: my * z -> q2u2 (m18)
                tt(M(18), M(16), M(28), AL.mult)
                # s4: mx * [y, mz*y] -> q1u4, q1u5 (m12, m13)
                tt(M(12, 2).rearrange("p (a s) -> p a s", a=2),
                   M(8, 2).rearrange("p (a s) -> p a s", a=2),
                   bcast(M(30), 2), AL.mult)
                # s5: mx * [z, my*z] -> q2u4(m20), q2u6(m22)
                d5 = MEGA[:, 20 * SEC:24 * SEC].rearrange(
                    "p (a b s) -> p a b s", a=2, b=2)[:, :, 0, :]
                s5 = MEGA[:, 16 * SEC:20 * SEC].rearrange(
                    "p (a b s) -> p a b s", a=2, b=2)[:, :, 0, :]
                tt(d5, s5, bcast(M(30), 2), AL.mult)

                # relus: x-block u4-7 <- relu(u0-3) (ACT)
                act(M(4, 4), M(0, 4), AF.Relu)
                yb = MEGA[:, 8 * SEC:16 * SEC].rearrange(
                    "p (h d s) -> p h d s", h=2, d=2)
                act(yb[:, :, 1, :], yb[:, :, 0, :], AF.Relu)
                zb = MEGA[:, 16 * SEC:24 * SEC].rearrange(
                    "p (h d s) -> p h d s", h=4, d=2)
                act(zb[:, :, 1, :], zb[:, :, 0, :], AF.Relu)

                # ---- k-reduction tree (all 34 arrays, DVE) ----
                MT = NM * TG  # 272 (m,t) rows
                T1 = mpool.tile([PART, MT * 32], FP16, name="T1")
                T2 = mpool.tile([PART, MT * 4], FP16, name="T2")
                mv = MEGA[:].rearrange("p (m h k) -> p m h k", m=MT, h=2)
                t1v = T1[:].rearrange("p (m k) -> p m k", m=MT)
                tt(t1v, mv[:, :, 0, :], mv[:, :, 1, :], AL.add)       # L1
                t1h = T1[:].rearrange("p (m h k) -> p m h k", m=MT, h=2)
                tt(t1h[:, :, 0, :], t1h[:, :, 0, :], t1h[:, :, 1, :],
                   AL.add)                                            # L2
                t1q = T1[:].rearrange("p (m h k) -> p m h k", m=MT, h=4)
                tt(t1q[:, :, 0, :], t1q[:, :, 0, :], t1q[:, :, 1, :],
                   AL.add)                                            # L3
                t1o = T1[:].rearrange("p (m h k) -> p m h k", m=MT, h=8)
                t2v = T2[:].rearrange("p (m k) -> p m k", m=MT)
                tt(t2v, t1o[:, :, 0, :], t1o[:, :, 1, :], AL.add)     # L4
                t2h = T2[:].rearrange("p (m h k) -> p m h k", m=MT, h=2)
                tt(t2h[:, :, 0, :], t2h[:, :, 0, :], t2h[:, :, 1, :],
                   AL.add)                                            # L5
                t2q = T2[:].rearrange("p (m t k) -> p m t k", m=NM, t=TG)
                tt(ep4[:, :, sl, :], t2q[:, :, :, 0], t2q[:, :, :, 1],
                   AL.add)                                            # L6

            # ---- batched epilogue (sum-space) ----
            act(CT[:, V32:8 * V32], EP[:, NMV * V32:NM * V32], AF.Copy)
            stv = EP[:, 0:24 * V32].rearrange(
                "p (c u v) -> p c u v", c=3, u=8)
            Q = epool.tile([PART, 3 * V32], F32, name="Q")
            act(Q[:].rearrange("p (c v) -> p c v", c=3),
                stv[:, :, 0, :], AF.Square, 0.0, 1.0 / 8.0)

            sub = nc.vector.tensor_tensor
            sub(stv[:, :, 0:4, :], stv[:, :, 0:4, :], stv[:, :, 4:8, :],
                AL.subtract)
            st5 = EP[:, 0:24 * V32].rearrange(
                "p (m u v) -> p m u v", m=6, u=4)
            sub(st5[:, :, 0:2, :], st5[:, :, 0:2, :], st5[:, :, 2:4, :],
                AL.subtract)
            st6 = EP[:, 0:24 * V32].rearrange(
                "p (m u v) -> p m u v", m=12, u=2)
            sub(st6[:, :, 0:1, :], st6[:, :, 0:1, :], st6[:, :, 1:2, :],
                AL.subtract)

            ct3 = CT[:].rearrange("p (u v) -> p u v", u=8)
            sub(ct3[:, 0:4, :], ct3[:, 0:4, :], ct3[:, 4:8, :], AL.subtract)
            ctr4 = CT[:].rearrange("p (a u v) -> p a u v", a=2, u=4)
            sub(ctr4[:, :, 0:2, :], ctr4[:, :, 0:2, :], ctr4[:, :, 2:4, :],
                AL.subtract)
            ctr5 = CT[:].rearrange("p (a u v) -> p a u v", a=4, u=2)
            sub(ctr5[:, :, 0:1, :], ctr5[:, :, 0:1, :], ctr5[:, :, 1:2, :],
                AL.subtract)

            CC = epool.tile([PART, 8 * V32], F32, name="CC")
            RC = epool.tile([PART, 8 * V32], F32, name="RC")
            nc.vector.tensor_scalar_max(CC[:], CT[:], 1.0)
            nc.vector.reciprocal_approx_fast(RC[:], CC[:])

            ovb = O[:].rearrange("p (s t f) -> p s t f", s=SB, t=TG)
            mn5 = ovb[:, :, :, 6:30].rearrange(
                "p s t (u c) -> p s t u c", u=8)
            rcv = RC[:].rearrange("p (u s t) -> p s t u", u=8, s=SB)
            for c in range(3):
                stc = stv[:, c].rearrange("p u (s t) -> p s t u", s=SB)
                nc.gpsimd.tensor_tensor(mn5[:, :, :, :, c], stc, rcv,
                                        AL.mult)

            # std = sqrt((SS - Q)/63)
            D = epool.tile([PART, 3 * V32], F32, name="D")
            sub(D[:], EP[:, 24 * V32:27 * V32], Q[:], AL.subtract)
            act(ovb[:, :, :, 0:3],
                D[:].rearrange("p (c s t) -> p s t c", c=3, s=SB),
                AF.Sqrt, 0.0, 1.0 / 63.0)
            nc.gpsimd.tensor_copy(
                ovb[:, :, :, 3:6],
                CIN[:].rearrange("p (s t c) -> p s t c", s=SB, t=TG))

            bb, nb0 = batch, 0
            nc.sync.dma_start(
                out=out[bb, nb0:nb0 + SB * SLAB, :].rearrange(
                    "(s p t) f -> p s t f", s=SB, p=PART),
                in_=ovb)


_CACHE: dict = {}


def _get_nc():
    if "nc" not in _CACHE:
        nc = bacc.Bacc("TRN2", target_bir_lowering=False, debug=False)
        _build_kernel(nc)
        nc.finalize()
        _CACHE["nc"] = nc
    return _CACHE["nc"]


def kernel(group_xyz: np.ndarray, new_xyz: np.ndarray) -> np.ndarray:
    nc = _get_nc()
    gx = np.ascontiguousarray(group_xyz, dtype=np.float32)
    nx = np.ascontiguousarray(new_xyz, dtype=np.float32)
    in_maps = [
        {"gx": gx[i * BL:(i + 1) * BL], "nx": nx[i * BL:(i + 1) * BL]}
        for i in range(NCORES)
    ]
    res = run_bass_kernel_spmd(nc, in_maps, list(range(NCORES)))
    return np.concatenate([res.results[i]["out"] for i in range(NCORES)],
                          axis=0)
